# revision 24
# baseline (speedup 1.0000x reference)
"""CartesianMACE rank-0 fused kernel for 8 trn2 NeuronCores — v6.

Math (per node n, 16x16 mats A=cw0[0,n], B=mw0[0,n], D=cw1[0,n],
E=mw1[0,n]; 16-vecs x=h0[n], m0=msg0_r0[n], m1=msg1_r0[n]):
    t = A x + B m0;  s[n] = colsum(D).t + colsum(E).m1
    out[c] = sum_n s[n] w_pred[c,n] + b_pred[c]
The reference's ranks 1/2 never reach the output, so their inputs are
unused.

Key design points (all HW-measured):
  - bf16 everywhere (gate is 2e-2; bf16 lands ~1e-3) -> 13.5 MB/core HBM.
  - ALL elementwise work on DVE: GpSimd tensor ops slow concurrent DVE
    ops 7-12x via the shared SBUF port, and SWDGE accumulate-DMA runs
    at only ~105 GB/s -- so GpSimd does nothing.
  - tensor_reduce runs 1x; TT-add fold trees run 2x for bf16 -> trees.
  - Supertiles processed in chunks of [1, 2, 2, 2]: doubled op sizes
    halve the ~150-cycle per-op overhead, and the first chunk stays
    small so the pipeline fills fast.
  - Per-node results accumulate into SBUF; one epilogue computes the
    head; per-core (128,2) partials summed on host (the hint's final
    all-reduce).

Sharding: data-parallel over nodes, 50000 padded to 50176 =
8 cores x 7 supertiles x 128 partitions x 7 groups.
"""

import sys

for _p in ("/opt/trn_rl_repo", "/root/.axon_site/_ro/trn_rl_repo"):
    if _p not in sys.path:
        sys.path.append(_p)

import numpy as np
import ml_dtypes

BF16 = ml_dtypes.bfloat16

N, CH = 50000, 16
CORES = 8
T, S = 7, 7
P = 128
GP = T * S
NP = CORES * T * 128 * S

_cache = {}
TRACE = False


def _split_multiwait(nc, mybir):
    """Split multi-wait sync info onto same-engine Drain carriers (this
    walrus build accepts a single wait per instruction)."""
    for fn in nc.m.functions:
        for bb in fn.blocks:
            insts = bb.instructions
            i = 0
            while i < len(insts):
                ins = insts[i]
                si = ins.sync_info
                if si is not None and len(si.on_wait) > 1:
                    waits = list(si.on_wait)
                    ins.sync_info = mybir.SyncInfo(
                        on_wait=waits[-1:], on_update=list(si.on_update))
                    for k, w in enumerate(waits[:-1]):
                        insts.insert(i + k, mybir.InstDrain(
                            name=f"{ins.name}_w{k}", opcode="Drain",
                            engine=ins.engine, ins=[], outs=[],
                            sync_info=mybir.SyncInfo(on_wait=[w], on_update=[]),
                        ))
                    i += len(waits) - 1
                i += 1


def _build_nc():
    import concourse.bass as bass
    import concourse.tile as tile
    import concourse.mybir as mybir

    f32 = mybir.dt.float32
    b16 = mybir.dt.bfloat16

    nc = bass.Bass("TRN2", target_bir_lowering=False, debug=False,
                   num_devices=CORES)

    # chunk 0 = supertile 0 alone; chunks 1-3 = supertile pairs with
    # groups interleaved as G=(t2, g): line layout
    # [AB (m j G k) | DE (q j G i)], 512*G elements per region
    mds_d = nc.dram_tensor("mds", [P, 7168], b16, kind="ExternalInput").ap()
    mdp_d = nc.dram_tensor("mdp", [3, P, 14336], b16,
                           kind="ExternalInput").ap()
    # aux: [st0: xm 224 | m1 112] + 3 pair blocks [xm' 448 | m1' 224]
    aux_d = nc.dram_tensor("aux", [P, 2352], b16, kind="ExternalInput").ap()
    w_d = nc.dram_tensor("w", [P, 2 * GP], f32, kind="ExternalInput").ap()
    o_d = nc.dram_tensor("o", [P, 2], f32, kind="ExternalOutput").ap()

    with tile.TileContext(nc) as tc:
        with (
            tc.tile_pool(name="mats", bufs=3) as mats,
            tc.tile_pool(name="work", bufs=2) as work,
            tc.tile_pool(name="acc", bufs=1) as acc,
        ):
            t_all = acc.tile([P, 2 * GP * 16], b16)  # (m, tg, j)
            d_all = acc.tile([P, 2 * GP * 16], b16)  # (q, tg, j)
            aux_sb = acc.tile([P, 2352], b16)
            w_sb = acc.tile([P, 2 * GP], f32)

            first = True
            tg0 = 0
            for ci, G in enumerate([S, 2 * S, 2 * S, 2 * S]):
                FABc = 2 * 16 * G * 16
                md_sb = mats.tile([P, 14336], b16, tag="md")
                if ci == 0:
                    nc.sync.dma_start(out=md_sb[:, 0:FABc], in_=mds_d[:, 0:FABc])
                    nc.sync.dma_start(out=md_sb[:, FABc:2 * FABc],
                                      in_=mds_d[:, FABc:2 * FABc])
                    xm_off = 0
                else:
                    nc.sync.dma_start(out=md_sb[:, 0:FABc],
                                      in_=mdp_d[ci - 1][:, 0:FABc])
                    nc.sync.dma_start(out=md_sb[:, FABc:2 * FABc],
                                      in_=mdp_d[ci - 1][:, FABc:2 * FABc])
                    xm_off = 336 + (ci - 1) * 672
                if first:
                    # aux/w after chunk0's transfers so compute starts asap
                    nc.sync.dma_start(out=aux_sb[:, :], in_=aux_d)
                    nc.sync.dma_start(out=w_sb[:, :], in_=w_d)
                    first = False

                ab = md_sb[:, 0:FABc].rearrange(
                    "p (m G j k) -> p m G j k", m=2, G=G, j=16, k=16)
                de = md_sb[:, FABc:2 * FABc].rearrange(
                    "p (q G j i) -> p q G j i", q=2, G=G, j=16, i=16)
                xm = aux_sb[:, xm_off:xm_off + 32 * G].rearrange(
                    "p (m G k) -> p m G k", m=2, G=G, k=16)

                # D/E first fold (i 16 -> 8)
                d1 = work.tile([P, 2 * 16 * 2 * S * 8], b16, tag="d1")
                d1v = d1[:, 0:2 * 16 * G * 8].rearrange(
                    "p (q G j i) -> p q G j i", q=2, G=G, j=16, i=8)
                nc.vector.tensor_add(out=d1v, in0=de[:, :, :, :, 0:8],
                                     in1=de[:, :, :, :, 8:16])

                # tmp[m,j,G,k] = {A,B}[j,G,k] * {x,m0}[G,k]  (one op, 2x)
                tmp = work.tile([P, 2 * 16 * 2 * S * 16], b16, tag="tmp")
                tmp5 = tmp[:, 0:FABc].rearrange("p (m G j k) -> p m G j k",
                                                m=2, G=G, j=16, k=16)
                nc.vector.tensor_mul(
                    out=tmp5, in0=ab,
                    in1=xm.unsqueeze(3).broadcast_to((P, 2, G, 16, 16)))

                # rowsum tree over k
                r1 = work.tile([P, 2 * 16 * 2 * S * 8], b16, tag="r1")
                r1v = r1[:, 0:2 * 16 * G * 8].rearrange(
                    "p (m G j k) -> p m G j k", m=2, G=G, j=16, k=8)
                nc.vector.tensor_add(out=r1v, in0=tmp5[:, :, :, :, 0:8],
                                     in1=tmp5[:, :, :, :, 8:16])
                r2 = work.tile([P, 2 * 16 * 2 * S * 4], b16, tag="r2")
                r2v = r2[:, 0:2 * 16 * G * 4].rearrange(
                    "p (m G j k) -> p m G j k", m=2, G=G, j=16, k=4)
                nc.vector.tensor_add(out=r2v, in0=r1v[:, :, :, :, 0:4],
                                     in1=r1v[:, :, :, :, 4:8])
                r3 = work.tile([P, 2 * 16 * 2 * S * 2], b16, tag="r3")
                r3v = r3[:, 0:2 * 16 * G * 2].rearrange(
                    "p (m G j k) -> p m G j k", m=2, G=G, j=16, k=2)
                nc.vector.tensor_add(out=r3v, in0=r2v[:, :, :, :, 0:2],
                                     in1=r2v[:, :, :, :, 2:4])
                tav = t_all[:, :].rearrange("p (m tg j) -> p m tg j",
                                            m=2, tg=GP, j=16)[:, :,
                                                              tg0:tg0 + G]
                nc.vector.tensor_add(out=tav, in0=r3v[:, :, :, :, 0],
                                     in1=r3v[:, :, :, :, 1])

                # colsum tree over i (8 -> 1)
                d2 = work.tile([P, 2 * 16 * 2 * S * 4], b16, tag="d2")
                d2v = d2[:, 0:2 * 16 * G * 4].rearrange(
                    "p (q G j i) -> p q G j i", q=2, G=G, j=16, i=4)
                nc.vector.tensor_add(out=d2v, in0=d1v[:, :, :, :, 0:4],
                                     in1=d1v[:, :, :, :, 4:8])
                d3 = work.tile([P, 2 * 16 * 2 * S * 2], b16, tag="d3")
                d3v = d3[:, 0:2 * 16 * G * 2].rearrange(
                    "p (q G j i) -> p q G j i", q=2, G=G, j=16, i=2)
                nc.vector.tensor_add(out=d3v, in0=d2v[:, :, :, :, 0:2],
                                     in1=d2v[:, :, :, :, 2:4])
                dav = d_all[:, :].rearrange("p (q tg j) -> p q tg j",
                                            q=2, tg=GP, j=16)[:, :,
                                                              tg0:tg0 + G]
                nc.vector.tensor_add(out=dav, in0=d3v[:, :, :, :, 0],
                                     in1=d3v[:, :, :, :, 1])
                tg0 += G

            # ---- epilogue ----
            H = GP * 16  # 784
            tab = acc.tile([P, H], b16)
            nc.vector.tensor_add(out=tab[:, :], in0=t_all[:, 0:H],
                                 in1=t_all[:, H:2 * H])
            pe = acc.tile([P, 2 * H], b16)
            nc.vector.tensor_mul(out=pe[:, 0:H], in0=tab[:, :],
                                 in1=d_all[:, 0:H])
            # em for st0 (m1 at aux[224:336], tg 0..6) then pairs
            nc.vector.tensor_mul(out=pe[:, H:H + 112],
                                 in0=d_all[:, H:H + 112],
                                 in1=aux_sb[:, 224:336])
            m1p = aux_sb[:, 336:2352].rearrange("p (pr blk) -> p pr blk",
                                                pr=3, blk=672)[:, :, 448:672]
            nc.vector.tensor_mul(
                out=pe[:, H + 112:2 * H].rearrange("p (pr x) -> p pr x",
                                                   pr=3, x=224),
                in0=d_all[:, H + 112:2 * H].rearrange("p (pr x) -> p pr x",
                                                      pr=3, x=224),
                in1=m1p)
            pev = pe[:, :].rearrange("p (qt g j) -> p qt g j",
                                     qt=2 * T, g=S, j=16)
            q1 = acc.tile([P, 2 * T * S * 8], b16)
            q1v = q1[:, :].rearrange("p (qt g j) -> p qt g j",
                                     qt=2 * T, g=S, j=8)
            nc.vector.tensor_add(out=q1v, in0=pev[:, :, :, 0:8],
                                 in1=pev[:, :, :, 8:16])
            q2 = acc.tile([P, 2 * T * S * 4], b16)
            q2v = q2[:, :].rearrange("p (qt g j) -> p qt g j",
                                     qt=2 * T, g=S, j=4)
            nc.vector.tensor_add(out=q2v, in0=q1v[:, :, :, 0:4],
                                 in1=q1v[:, :, :, 4:8])
            q3 = acc.tile([P, 2 * T * S * 2], b16)
            q3v = q3[:, :].rearrange("p (qt g j) -> p qt g j",
                                     qt=2 * T, g=S, j=2)
            nc.vector.tensor_add(out=q3v, in0=q2v[:, :, :, 0:2],
                                 in1=q2v[:, :, :, 2:4])
            sq = acc.tile([P, 2 * GP], f32)
            nc.vector.tensor_add(
                out=sq[:, :].rearrange("p (qt g) -> p qt g", qt=2 * T, g=S),
                in0=q3v[:, :, :, 0], in1=q3v[:, :, :, 1])
            s = acc.tile([P, GP], f32)
            nc.vector.tensor_add(out=s[:, :], in0=sq[:, 0:GP],
                                 in1=sq[:, GP:2 * GP])
            junk = acc.tile([P, 2 * GP], f32)
            nc.vector.tensor_mul(
                out=junk[:, :].rearrange("p (c tg) -> p c tg", c=2, tg=GP),
                in0=s[:, :].rearrange("p tg -> p tg").unsqueeze(1)
                .broadcast_to((P, 2, GP)),
                in1=w_sb[:, :].rearrange("p (c tg) -> p c tg", c=2, tg=GP))
            o_sb = acc.tile([P, 2], f32)
            nc.vector.reduce_sum(
                out=o_sb[:, :].rearrange("p c -> p c"),
                in_=junk[:, :].rearrange("p (c tg) -> p c tg", c=2, tg=GP),
                axis=mybir.AxisListType.X)
            nc.sync.dma_start(out=o_d, in_=o_sb[:, :])

    return nc


def _get_nc():
    if "nc" not in _cache:
        _cache["nc"] = _build_nc()
    return _cache["nc"]


def _shard6(m):
    """(N,16,16) f32 -> (CORES, T, 128, S, 16, 16) [c,t,p,g,r,c2] padded."""
    out = np.zeros((NP, 16, 16), np.float32)
    out[:N] = np.asarray(m, np.float32)
    return out.reshape(CORES, T, 128, S, 16, 16)


def _shard_vec(v):
    out = np.zeros((NP, 16), np.float32)
    out[:N] = np.asarray(v, np.float32).reshape(N, 16)
    return out.reshape(CORES, T, 128, S, 16)


def kernel(h0, cw0, mw0, cw1, mw1,
           msg0_r0, msg0_r1, msg0_r2,
           msg1_r0, msg1_r1, msg1_r2,
           w_pred, b_pred):
    from concourse.bass_utils import run_bass_kernel_spmd

    nc = _get_nc()
    if not _cache.get("split_done"):
        import concourse.mybir as mybir
        _split_multiwait(nc, mybir)
        _cache["split_done"] = True

    # md: [c, t, p, reg(2: AB|DE), m/q(2), g(7), j(16), k/i(16)]
    md = np.empty((CORES, T, 128, 2, 2, S, 16, 16), BF16)
    md[:, :, :, 0, 0] = _shard6(cw0[0])
    md[:, :, :, 0, 1] = _shard6(mw0[0])
    md[:, :, :, 1, 0] = _shard6(cw1[0]).transpose(0, 1, 2, 3, 5, 4)
    md[:, :, :, 1, 1] = _shard6(mw1[0]).transpose(0, 1, 2, 3, 5, 4)
    MDS = np.ascontiguousarray(md[:, 0].reshape(CORES, 128, 7168))
    # pairs: G=(t2, g) -> [c, pr, p, reg, m, t2, g, j, k]
    MDP = np.ascontiguousarray(
        md[:, 1:7].reshape(CORES, 3, 2, 128, 2, 2, S, 16, 16)
        .transpose(0, 1, 3, 4, 5, 2, 6, 7, 8).reshape(CORES, 3, 128, 14336))

    X = _shard_vec(np.asarray(h0, np.float32)[..., 0])        # c t p g k
    M0 = _shard_vec(np.asarray(msg0_r0, np.float32)[..., 0])
    M1 = _shard_vec(np.asarray(msg1_r0, np.float32)[..., 0])
    aux = np.empty((CORES, 128, 2352), BF16)
    aux[:, :, 0:112] = X[:, 0].reshape(CORES, 128, 112)
    aux[:, :, 112:224] = M0[:, 0].reshape(CORES, 128, 112)
    aux[:, :, 224:336] = M1[:, 0].reshape(CORES, 128, 112)
    XP = X[:, 1:7].reshape(CORES, 3, 2, 128, 112).transpose(0, 3, 1, 2, 4)
    M0P = M0[:, 1:7].reshape(CORES, 3, 2, 128, 112).transpose(0, 3, 1, 2, 4)
    M1P = M1[:, 1:7].reshape(CORES, 3, 2, 128, 112).transpose(0, 3, 1, 2, 4)
    blocks = np.concatenate(
        [np.stack([XP, M0P], axis=3).reshape(CORES, 128, 3, 448),
         M1P.reshape(CORES, 128, 3, 224)], axis=3)        # [c, p, 3, 672]
    aux[:, :, 336:2352] = blocks.reshape(CORES, 128, 2016)
    AUX = np.ascontiguousarray(aux)

    wp = np.zeros((2, NP), np.float32)
    wp[:, :N] = np.asarray(w_pred, np.float32)
    W = np.ascontiguousarray(
        wp.reshape(2, CORES, T, 128, S).transpose(1, 3, 0, 2, 4)
        .reshape(CORES, 128, 2 * GP))

    in_maps = [
        {"mds": MDS[i], "mdp": MDP[i], "aux": AUX[i], "w": W[i]}
        for i in range(CORES)
    ]
    res = run_bass_kernel_spmd(nc, in_maps, list(range(CORES)), trace=TRACE)
    _cache["last_res"] = res
    partial = np.zeros(2, np.float64)
    for i in range(CORES):
        partial += res.results[i]["o"].astype(np.float64).sum(axis=0)
    out = (partial + np.asarray(b_pred, np.float64)).astype(np.float32)
    return out.reshape(1, 2)


# revision 25
# speedup vs baseline: 1.1380x; 1.1380x over previous
"""CartesianMACE rank-0 fused kernel for 8 trn2 NeuronCores — v6.

Math (per node n, 16x16 mats A=cw0[0,n], B=mw0[0,n], D=cw1[0,n],
E=mw1[0,n]; 16-vecs x=h0[n], m0=msg0_r0[n], m1=msg1_r0[n]):
    t = A x + B m0;  s[n] = colsum(D).t + colsum(E).m1
    out[c] = sum_n s[n] w_pred[c,n] + b_pred[c]
The reference's ranks 1/2 never reach the output, so their inputs are
unused.

Key design points (all HW-measured):
  - bf16 everywhere (gate is 2e-2; bf16 lands ~1e-3) -> 13.5 MB/core HBM.
  - ALL elementwise work on DVE: GpSimd tensor ops slow concurrent DVE
    ops 7-12x via the shared SBUF port, and SWDGE accumulate-DMA runs
    at only ~105 GB/s -- so GpSimd does nothing.
  - tensor_reduce runs 1x; TT-add fold trees run 2x for bf16 -> trees.
  - Supertiles processed in chunks of [1, 2, 2, 2]: doubled op sizes
    halve the ~150-cycle per-op overhead, and the first chunk stays
    small so the pipeline fills fast.
  - Per-node results accumulate into SBUF; one epilogue computes the
    head; per-core (128,2) partials summed on host (the hint's final
    all-reduce).

Sharding: data-parallel over nodes, 50000 padded to 50176 =
8 cores x 7 supertiles x 128 partitions x 7 groups.
"""

import sys

for _p in ("/opt/trn_rl_repo", "/root/.axon_site/_ro/trn_rl_repo"):
    if _p not in sys.path:
        sys.path.append(_p)

import numpy as np
import ml_dtypes

BF16 = ml_dtypes.bfloat16

N, CH = 50000, 16
CORES = 8
T, S = 7, 7
P = 128
GP = T * S
NP = CORES * T * 128 * S

_cache = {}
TRACE = False


def _split_multiwait(nc, mybir):
    """Split multi-wait sync info onto same-engine Drain carriers (this
    walrus build accepts a single wait per instruction)."""
    for fn in nc.m.functions:
        for bb in fn.blocks:
            insts = bb.instructions
            i = 0
            while i < len(insts):
                ins = insts[i]
                si = ins.sync_info
                if si is not None and len(si.on_wait) > 1:
                    waits = list(si.on_wait)
                    ins.sync_info = mybir.SyncInfo(
                        on_wait=waits[-1:], on_update=list(si.on_update))
                    for k, w in enumerate(waits[:-1]):
                        insts.insert(i + k, mybir.InstDrain(
                            name=f"{ins.name}_w{k}", opcode="Drain",
                            engine=ins.engine, ins=[], outs=[],
                            sync_info=mybir.SyncInfo(on_wait=[w], on_update=[]),
                        ))
                    i += len(waits) - 1
                i += 1


def _build_nc():
    import concourse.bass as bass
    import concourse.tile as tile
    import concourse.mybir as mybir

    f32 = mybir.dt.float32
    b16 = mybir.dt.bfloat16

    nc = bass.Bass("TRN2", target_bir_lowering=False, debug=False,
                   num_devices=CORES)

    # chunk 0 = supertile 0 alone; chunks 1-3 = supertile pairs with
    # groups interleaved as G=(t2, g): line layout
    # [AB (m j G k) | DE (q j G i)], 512*G elements per region
    mds_d = nc.dram_tensor("mds", [P, 7168], b16, kind="ExternalInput").ap()
    mdp_d = nc.dram_tensor("mdp", [3, P, 14336], b16,
                           kind="ExternalInput").ap()
    # aux: [st0: xm 224 | m1 112] + 3 pair blocks [xm' 448 | m1' 224]
    aux_d = nc.dram_tensor("aux", [P, 2352], b16, kind="ExternalInput").ap()
    w_d = nc.dram_tensor("w", [P, 2 * GP], f32, kind="ExternalInput").ap()
    o_d = nc.dram_tensor("o", [P, 2], f32, kind="ExternalOutput").ap()

    with tile.TileContext(nc) as tc:
        with (
            tc.tile_pool(name="mats", bufs=3) as mats,
            tc.tile_pool(name="work", bufs=2) as work,
            tc.tile_pool(name="acc", bufs=1) as acc,
        ):
            t_all = acc.tile([P, 2 * GP * 16], b16)  # (m, tg, j)
            d_all = acc.tile([P, 2 * GP * 16], b16)  # (q, tg, j)
            aux_sb = acc.tile([P, 2352], b16)
            w_sb = acc.tile([P, 2 * GP], f32)

            first = True
            tg0 = 0
            for ci, G in enumerate([S, 2 * S, 2 * S, 2 * S]):
                FABc = 2 * 16 * G * 16
                md_sb = mats.tile([P, 14336], b16, tag="md")
                if ci == 0:
                    # DE first: the chunk's first compute op (d1) reads it
                    nc.sync.dma_start(out=md_sb[:, FABc:2 * FABc],
                                      in_=mds_d[:, FABc:2 * FABc])
                    nc.sync.dma_start(out=md_sb[:, 0:FABc], in_=mds_d[:, 0:FABc])
                    xm_off = 0
                else:
                    nc.sync.dma_start(out=md_sb[:, FABc:2 * FABc],
                                      in_=mdp_d[ci - 1][:, FABc:2 * FABc])
                    nc.sync.dma_start(out=md_sb[:, 0:FABc],
                                      in_=mdp_d[ci - 1][:, 0:FABc])
                    xm_off = 336 + (ci - 1) * 672
                if first:
                    # aux/w after chunk0's transfers so compute starts asap
                    nc.sync.dma_start(out=aux_sb[:, :], in_=aux_d)
                    nc.sync.dma_start(out=w_sb[:, :], in_=w_d)
                    first = False

                ab = md_sb[:, 0:FABc].rearrange(
                    "p (m G j k) -> p m G j k", m=2, G=G, j=16, k=16)
                de = md_sb[:, FABc:2 * FABc].rearrange(
                    "p (q G j i) -> p q G j i", q=2, G=G, j=16, i=16)
                xm = aux_sb[:, xm_off:xm_off + 32 * G].rearrange(
                    "p (m G k) -> p m G k", m=2, G=G, k=16)

                # D/E first fold (i 16 -> 8)
                d1 = work.tile([P, 2 * 16 * 2 * S * 8], b16, tag="d1")
                d1v = d1[:, 0:2 * 16 * G * 8].rearrange(
                    "p (q G j i) -> p q G j i", q=2, G=G, j=16, i=8)
                nc.vector.tensor_add(out=d1v, in0=de[:, :, :, :, 0:8],
                                     in1=de[:, :, :, :, 8:16])

                # tmp[m,j,G,k] = {A,B}[j,G,k] * {x,m0}[G,k]  (one op, 2x)
                tmp = work.tile([P, 2 * 16 * 2 * S * 16], b16, tag="tmp")
                tmp5 = tmp[:, 0:FABc].rearrange("p (m G j k) -> p m G j k",
                                                m=2, G=G, j=16, k=16)
                nc.vector.tensor_mul(
                    out=tmp5, in0=ab,
                    in1=xm.unsqueeze(3).broadcast_to((P, 2, G, 16, 16)))

                # rowsum tree over k
                r1 = work.tile([P, 2 * 16 * 2 * S * 8], b16, tag="r1")
                r1v = r1[:, 0:2 * 16 * G * 8].rearrange(
                    "p (m G j k) -> p m G j k", m=2, G=G, j=16, k=8)
                nc.vector.tensor_add(out=r1v, in0=tmp5[:, :, :, :, 0:8],
                                     in1=tmp5[:, :, :, :, 8:16])
                r2 = work.tile([P, 2 * 16 * 2 * S * 4], b16, tag="r2")
                r2v = r2[:, 0:2 * 16 * G * 4].rearrange(
                    "p (m G j k) -> p m G j k", m=2, G=G, j=16, k=4)
                nc.vector.tensor_add(out=r2v, in0=r1v[:, :, :, :, 0:4],
                                     in1=r1v[:, :, :, :, 4:8])
                r3 = work.tile([P, 2 * 16 * 2 * S * 2], b16, tag="r3")
                r3v = r3[:, 0:2 * 16 * G * 2].rearrange(
                    "p (m G j k) -> p m G j k", m=2, G=G, j=16, k=2)
                nc.vector.tensor_add(out=r3v, in0=r2v[:, :, :, :, 0:2],
                                     in1=r2v[:, :, :, :, 2:4])
                tav = t_all[:, :].rearrange("p (m tg j) -> p m tg j",
                                            m=2, tg=GP, j=16)[:, :,
                                                              tg0:tg0 + G]
                nc.vector.tensor_add(out=tav, in0=r3v[:, :, :, :, 0],
                                     in1=r3v[:, :, :, :, 1])

                # colsum tree over i (8 -> 1)
                d2 = work.tile([P, 2 * 16 * 2 * S * 4], b16, tag="d2")
                d2v = d2[:, 0:2 * 16 * G * 4].rearrange(
                    "p (q G j i) -> p q G j i", q=2, G=G, j=16, i=4)
                nc.vector.tensor_add(out=d2v, in0=d1v[:, :, :, :, 0:4],
                                     in1=d1v[:, :, :, :, 4:8])
                d3 = work.tile([P, 2 * 16 * 2 * S * 2], b16, tag="d3")
                d3v = d3[:, 0:2 * 16 * G * 2].rearrange(
                    "p (q G j i) -> p q G j i", q=2, G=G, j=16, i=2)
                nc.vector.tensor_add(out=d3v, in0=d2v[:, :, :, :, 0:2],
                                     in1=d2v[:, :, :, :, 2:4])
                dav = d_all[:, :].rearrange("p (q tg j) -> p q tg j",
                                            q=2, tg=GP, j=16)[:, :,
                                                              tg0:tg0 + G]
                nc.vector.tensor_add(out=dav, in0=d3v[:, :, :, :, 0],
                                     in1=d3v[:, :, :, :, 1])
                tg0 += G

            # ---- epilogue ----
            H = GP * 16  # 784
            tab = acc.tile([P, H], b16)
            nc.vector.tensor_add(out=tab[:, :], in0=t_all[:, 0:H],
                                 in1=t_all[:, H:2 * H])
            pe = acc.tile([P, 2 * H], b16)
            nc.vector.tensor_mul(out=pe[:, 0:H], in0=tab[:, :],
                                 in1=d_all[:, 0:H])
            # em for st0 (m1 at aux[224:336], tg 0..6) then pairs
            nc.vector.tensor_mul(out=pe[:, H:H + 112],
                                 in0=d_all[:, H:H + 112],
                                 in1=aux_sb[:, 224:336])
            m1p = aux_sb[:, 336:2352].rearrange("p (pr blk) -> p pr blk",
                                                pr=3, blk=672)[:, :, 448:672]
            nc.vector.tensor_mul(
                out=pe[:, H + 112:2 * H].rearrange("p (pr x) -> p pr x",
                                                   pr=3, x=224),
                in0=d_all[:, H + 112:2 * H].rearrange("p (pr x) -> p pr x",
                                                      pr=3, x=224),
                in1=m1p)
            pev = pe[:, :].rearrange("p (qt g j) -> p qt g j",
                                     qt=2 * T, g=S, j=16)
            q1 = acc.tile([P, 2 * T * S * 8], b16)
            q1v = q1[:, :].rearrange("p (qt g j) -> p qt g j",
                                     qt=2 * T, g=S, j=8)
            nc.vector.tensor_add(out=q1v, in0=pev[:, :, :, 0:8],
                                 in1=pev[:, :, :, 8:16])
            q2 = acc.tile([P, 2 * T * S * 4], b16)
            q2v = q2[:, :].rearrange("p (qt g j) -> p qt g j",
                                     qt=2 * T, g=S, j=4)
            nc.vector.tensor_add(out=q2v, in0=q1v[:, :, :, 0:4],
                                 in1=q1v[:, :, :, 4:8])
            q3 = acc.tile([P, 2 * T * S * 2], b16)
            q3v = q3[:, :].rearrange("p (qt g j) -> p qt g j",
                                     qt=2 * T, g=S, j=2)
            nc.vector.tensor_add(out=q3v, in0=q2v[:, :, :, 0:2],
                                 in1=q2v[:, :, :, 2:4])
            sq = acc.tile([P, 2 * GP], f32)
            nc.vector.tensor_add(
                out=sq[:, :].rearrange("p (qt g) -> p qt g", qt=2 * T, g=S),
                in0=q3v[:, :, :, 0], in1=q3v[:, :, :, 1])
            s = acc.tile([P, GP], f32)
            nc.vector.tensor_add(out=s[:, :], in0=sq[:, 0:GP],
                                 in1=sq[:, GP:2 * GP])
            junk = acc.tile([P, 2 * GP], f32)
            nc.vector.tensor_mul(
                out=junk[:, :].rearrange("p (c tg) -> p c tg", c=2, tg=GP),
                in0=s[:, :].rearrange("p tg -> p tg").unsqueeze(1)
                .broadcast_to((P, 2, GP)),
                in1=w_sb[:, :].rearrange("p (c tg) -> p c tg", c=2, tg=GP))
            o_sb = acc.tile([P, 2], f32)
            nc.vector.reduce_sum(
                out=o_sb[:, :].rearrange("p c -> p c"),
                in_=junk[:, :].rearrange("p (c tg) -> p c tg", c=2, tg=GP),
                axis=mybir.AxisListType.X)
            nc.sync.dma_start(out=o_d, in_=o_sb[:, :])

    return nc


def _get_nc():
    if "nc" not in _cache:
        _cache["nc"] = _build_nc()
    return _cache["nc"]


def _shard6(m):
    """(N,16,16) f32 -> (CORES, T, 128, S, 16, 16) [c,t,p,g,r,c2] padded."""
    out = np.zeros((NP, 16, 16), np.float32)
    out[:N] = np.asarray(m, np.float32)
    return out.reshape(CORES, T, 128, S, 16, 16)


def _shard_vec(v):
    out = np.zeros((NP, 16), np.float32)
    out[:N] = np.asarray(v, np.float32).reshape(N, 16)
    return out.reshape(CORES, T, 128, S, 16)


def kernel(h0, cw0, mw0, cw1, mw1,
           msg0_r0, msg0_r1, msg0_r2,
           msg1_r0, msg1_r1, msg1_r2,
           w_pred, b_pred):
    from concourse.bass_utils import run_bass_kernel_spmd

    nc = _get_nc()
    if not _cache.get("split_done"):
        import concourse.mybir as mybir
        _split_multiwait(nc, mybir)
        _cache["split_done"] = True

    # md: [c, t, p, reg(2: AB|DE), m/q(2), g(7), j(16), k/i(16)]
    md = np.empty((CORES, T, 128, 2, 2, S, 16, 16), BF16)
    md[:, :, :, 0, 0] = _shard6(cw0[0])
    md[:, :, :, 0, 1] = _shard6(mw0[0])
    md[:, :, :, 1, 0] = _shard6(cw1[0]).transpose(0, 1, 2, 3, 5, 4)
    md[:, :, :, 1, 1] = _shard6(mw1[0]).transpose(0, 1, 2, 3, 5, 4)
    MDS = np.ascontiguousarray(md[:, 0].reshape(CORES, 128, 7168))
    # pairs: G=(t2, g) -> [c, pr, p, reg, m, t2, g, j, k]
    MDP = np.ascontiguousarray(
        md[:, 1:7].reshape(CORES, 3, 2, 128, 2, 2, S, 16, 16)
        .transpose(0, 1, 3, 4, 5, 2, 6, 7, 8).reshape(CORES, 3, 128, 14336))

    X = _shard_vec(np.asarray(h0, np.float32)[..., 0])        # c t p g k
    M0 = _shard_vec(np.asarray(msg0_r0, np.float32)[..., 0])
    M1 = _shard_vec(np.asarray(msg1_r0, np.float32)[..., 0])
    aux = np.empty((CORES, 128, 2352), BF16)
    aux[:, :, 0:112] = X[:, 0].reshape(CORES, 128, 112)
    aux[:, :, 112:224] = M0[:, 0].reshape(CORES, 128, 112)
    aux[:, :, 224:336] = M1[:, 0].reshape(CORES, 128, 112)
    XP = X[:, 1:7].reshape(CORES, 3, 2, 128, 112).transpose(0, 3, 1, 2, 4)
    M0P = M0[:, 1:7].reshape(CORES, 3, 2, 128, 112).transpose(0, 3, 1, 2, 4)
    M1P = M1[:, 1:7].reshape(CORES, 3, 2, 128, 112).transpose(0, 3, 1, 2, 4)
    blocks = np.concatenate(
        [np.stack([XP, M0P], axis=3).reshape(CORES, 128, 3, 448),
         M1P.reshape(CORES, 128, 3, 224)], axis=3)        # [c, p, 3, 672]
    aux[:, :, 336:2352] = blocks.reshape(CORES, 128, 2016)
    AUX = np.ascontiguousarray(aux)

    wp = np.zeros((2, NP), np.float32)
    wp[:, :N] = np.asarray(w_pred, np.float32)
    W = np.ascontiguousarray(
        wp.reshape(2, CORES, T, 128, S).transpose(1, 3, 0, 2, 4)
        .reshape(CORES, 128, 2 * GP))

    in_maps = [
        {"mds": MDS[i], "mdp": MDP[i], "aux": AUX[i], "w": W[i]}
        for i in range(CORES)
    ]
    res = run_bass_kernel_spmd(nc, in_maps, list(range(CORES)), trace=TRACE)
    _cache["last_res"] = res
    partial = np.zeros(2, np.float64)
    for i in range(CORES):
        partial += res.results[i]["o"].astype(np.float64).sum(axis=0)
    out = (partial + np.asarray(b_pred, np.float64)).astype(np.float32)
    return out.reshape(1, 2)


# revision 26
# speedup vs baseline: 1.1870x; 1.0430x over previous
"""CartesianMACE rank-0 fused kernel for 8 trn2 NeuronCores — v6.

Math (per node n, 16x16 mats A=cw0[0,n], B=mw0[0,n], D=cw1[0,n],
E=mw1[0,n]; 16-vecs x=h0[n], m0=msg0_r0[n], m1=msg1_r0[n]):
    t = A x + B m0;  s[n] = colsum(D).t + colsum(E).m1
    out[c] = sum_n s[n] w_pred[c,n] + b_pred[c]
The reference's ranks 1/2 never reach the output, so their inputs are
unused.

Key design points (all HW-measured):
  - bf16 everywhere (gate is 2e-2; bf16 lands ~1e-3) -> 13.5 MB/core HBM.
  - ALL elementwise work on DVE: GpSimd tensor ops slow concurrent DVE
    ops 7-12x via the shared SBUF port, and SWDGE accumulate-DMA runs
    at only ~105 GB/s -- so GpSimd does nothing.
  - tensor_reduce runs 1x; TT-add fold trees run 2x for bf16 -> trees.
  - Supertiles processed in chunks of [1, 2, 2, 2]: doubled op sizes
    halve the ~150-cycle per-op overhead, and the first chunk stays
    small so the pipeline fills fast.
  - Per-node results accumulate into SBUF; one epilogue computes the
    head; per-core (128,2) partials summed on host (the hint's final
    all-reduce).

Sharding: data-parallel over nodes, 50000 padded to 50176 =
8 cores x 7 supertiles x 128 partitions x 7 groups.
"""

import sys

for _p in ("/opt/trn_rl_repo", "/root/.axon_site/_ro/trn_rl_repo"):
    if _p not in sys.path:
        sys.path.append(_p)

import numpy as np
import ml_dtypes

BF16 = ml_dtypes.bfloat16

N, CH = 50000, 16
CORES = 8
T, S = 7, 7
P = 128
GP = T * S
NP = CORES * T * 128 * S

_cache = {}
TRACE = False


def _split_multiwait(nc, mybir):
    """Split multi-wait sync info onto same-engine Drain carriers (this
    walrus build accepts a single wait per instruction)."""
    for fn in nc.m.functions:
        for bb in fn.blocks:
            insts = bb.instructions
            i = 0
            while i < len(insts):
                ins = insts[i]
                si = ins.sync_info
                if si is not None and len(si.on_wait) > 1:
                    waits = list(si.on_wait)
                    ins.sync_info = mybir.SyncInfo(
                        on_wait=waits[-1:], on_update=list(si.on_update))
                    for k, w in enumerate(waits[:-1]):
                        insts.insert(i + k, mybir.InstDrain(
                            name=f"{ins.name}_w{k}", opcode="Drain",
                            engine=ins.engine, ins=[], outs=[],
                            sync_info=mybir.SyncInfo(on_wait=[w], on_update=[]),
                        ))
                    i += len(waits) - 1
                i += 1


def _build_nc():
    import concourse.bass as bass
    import concourse.tile as tile
    import concourse.mybir as mybir

    f32 = mybir.dt.float32
    b16 = mybir.dt.bfloat16

    nc = bass.Bass("TRN2", target_bir_lowering=False, debug=False,
                   num_devices=CORES)

    # chunk 0 = supertile 0 alone; chunks 1-3 = supertile pairs with
    # groups interleaved as G=(t2, g): line layout
    # [AB (m j G k) | DE (q j G i)], 512*G elements per region
    mds_d = nc.dram_tensor("mds", [P, 7168], b16, kind="ExternalInput").ap()
    mdp_d = nc.dram_tensor("mdp", [3, P, 14336], b16,
                           kind="ExternalInput").ap()
    # aux: [st0: xm 224 | m1 112] + 3 pair blocks [xm' 448 | m1' 224]
    aux_d = nc.dram_tensor("aux", [P, 2352], b16, kind="ExternalInput").ap()
    w_d = nc.dram_tensor("w", [P, 2 * GP], f32, kind="ExternalInput").ap()
    o_d = nc.dram_tensor("o", [P, 2], f32, kind="ExternalOutput").ap()

    with tile.TileContext(nc) as tc:
        with (
            tc.tile_pool(name="mats", bufs=3) as mats,
            tc.tile_pool(name="work", bufs=2) as work,
            tc.tile_pool(name="acc", bufs=1) as acc,
        ):
            t_all = acc.tile([P, 2 * GP * 16], b16)  # (m, tg, j)
            d_all = acc.tile([P, 2 * GP * 16], b16)  # (q, tg, j)
            aux_sb = acc.tile([P, 2352], b16)
            w_sb = acc.tile([P, 2 * GP], f32)

            first = True
            tg0 = 0
            for ci, G in enumerate([S, 2 * S, 2 * S, 2 * S]):
                FABc = 2 * 16 * G * 16
                md_sb = mats.tile([P, 14336], b16, tag="md")
                if ci == 0:
                    # DE first (d1 reads it), then aux (mulAB reads it),
                    # then AB -- early deps land in consumption order
                    nc.sync.dma_start(out=md_sb[:, FABc:2 * FABc],
                                      in_=mds_d[:, FABc:2 * FABc])
                    nc.sync.dma_start(out=aux_sb[:, :], in_=aux_d)
                    nc.sync.dma_start(out=md_sb[:, 0:FABc], in_=mds_d[:, 0:FABc])
                    xm_off = 0
                else:
                    nc.sync.dma_start(out=md_sb[:, FABc:2 * FABc],
                                      in_=mdp_d[ci - 1][:, FABc:2 * FABc])
                    nc.sync.dma_start(out=md_sb[:, 0:FABc],
                                      in_=mdp_d[ci - 1][:, 0:FABc])
                    xm_off = 336 + (ci - 1) * 672
                if first:
                    nc.sync.dma_start(out=w_sb[:, :], in_=w_d)
                    first = False

                ab = md_sb[:, 0:FABc].rearrange(
                    "p (m G j k) -> p m G j k", m=2, G=G, j=16, k=16)
                de = md_sb[:, FABc:2 * FABc].rearrange(
                    "p (q G j i) -> p q G j i", q=2, G=G, j=16, i=16)
                xm = aux_sb[:, xm_off:xm_off + 32 * G].rearrange(
                    "p (m G k) -> p m G k", m=2, G=G, k=16)

                # D/E first fold (i 16 -> 8)
                d1 = work.tile([P, 2 * 16 * 2 * S * 8], b16, tag="d1")
                d1v = d1[:, 0:2 * 16 * G * 8].rearrange(
                    "p (q G j i) -> p q G j i", q=2, G=G, j=16, i=8)
                nc.vector.tensor_add(out=d1v, in0=de[:, :, :, :, 0:8],
                                     in1=de[:, :, :, :, 8:16])

                # tmp[m,j,G,k] = {A,B}[j,G,k] * {x,m0}[G,k]  (one op, 2x)
                tmp = work.tile([P, 2 * 16 * 2 * S * 16], b16, tag="tmp")
                tmp5 = tmp[:, 0:FABc].rearrange("p (m G j k) -> p m G j k",
                                                m=2, G=G, j=16, k=16)
                nc.vector.tensor_mul(
                    out=tmp5, in0=ab,
                    in1=xm.unsqueeze(3).broadcast_to((P, 2, G, 16, 16)))

                # rowsum tree over k
                r1 = work.tile([P, 2 * 16 * 2 * S * 8], b16, tag="r1")
                r1v = r1[:, 0:2 * 16 * G * 8].rearrange(
                    "p (m G j k) -> p m G j k", m=2, G=G, j=16, k=8)
                nc.vector.tensor_add(out=r1v, in0=tmp5[:, :, :, :, 0:8],
                                     in1=tmp5[:, :, :, :, 8:16])
                r2 = work.tile([P, 2 * 16 * 2 * S * 4], b16, tag="r2")
                r2v = r2[:, 0:2 * 16 * G * 4].rearrange(
                    "p (m G j k) -> p m G j k", m=2, G=G, j=16, k=4)
                nc.vector.tensor_add(out=r2v, in0=r1v[:, :, :, :, 0:4],
                                     in1=r1v[:, :, :, :, 4:8])
                r3 = work.tile([P, 2 * 16 * 2 * S * 2], b16, tag="r3")
                r3v = r3[:, 0:2 * 16 * G * 2].rearrange(
                    "p (m G j k) -> p m G j k", m=2, G=G, j=16, k=2)
                nc.vector.tensor_add(out=r3v, in0=r2v[:, :, :, :, 0:2],
                                     in1=r2v[:, :, :, :, 2:4])
                tav = t_all[:, :].rearrange("p (m tg j) -> p m tg j",
                                            m=2, tg=GP, j=16)[:, :,
                                                              tg0:tg0 + G]
                nc.vector.tensor_add(out=tav, in0=r3v[:, :, :, :, 0],
                                     in1=r3v[:, :, :, :, 1])

                # colsum tree over i (8 -> 1)
                d2 = work.tile([P, 2 * 16 * 2 * S * 4], b16, tag="d2")
                d2v = d2[:, 0:2 * 16 * G * 4].rearrange(
                    "p (q G j i) -> p q G j i", q=2, G=G, j=16, i=4)
                nc.vector.tensor_add(out=d2v, in0=d1v[:, :, :, :, 0:4],
                                     in1=d1v[:, :, :, :, 4:8])
                d3 = work.tile([P, 2 * 16 * 2 * S * 2], b16, tag="d3")
                d3v = d3[:, 0:2 * 16 * G * 2].rearrange(
                    "p (q G j i) -> p q G j i", q=2, G=G, j=16, i=2)
                nc.vector.tensor_add(out=d3v, in0=d2v[:, :, :, :, 0:2],
                                     in1=d2v[:, :, :, :, 2:4])
                dav = d_all[:, :].rearrange("p (q tg j) -> p q tg j",
                                            q=2, tg=GP, j=16)[:, :,
                                                              tg0:tg0 + G]
                nc.vector.tensor_add(out=dav, in0=d3v[:, :, :, :, 0],
                                     in1=d3v[:, :, :, :, 1])
                tg0 += G

            # ---- epilogue ----
            H = GP * 16  # 784
            tab = acc.tile([P, H], b16)
            nc.vector.tensor_add(out=tab[:, :], in0=t_all[:, 0:H],
                                 in1=t_all[:, H:2 * H])
            pe = acc.tile([P, 2 * H], b16)
            nc.vector.tensor_mul(out=pe[:, 0:H], in0=tab[:, :],
                                 in1=d_all[:, 0:H])
            # em for st0 (m1 at aux[224:336], tg 0..6) then pairs
            nc.vector.tensor_mul(out=pe[:, H:H + 112],
                                 in0=d_all[:, H:H + 112],
                                 in1=aux_sb[:, 224:336])
            m1p = aux_sb[:, 336:2352].rearrange("p (pr blk) -> p pr blk",
                                                pr=3, blk=672)[:, :, 448:672]
            nc.vector.tensor_mul(
                out=pe[:, H + 112:2 * H].rearrange("p (pr x) -> p pr x",
                                                   pr=3, x=224),
                in0=d_all[:, H + 112:2 * H].rearrange("p (pr x) -> p pr x",
                                                      pr=3, x=224),
                in1=m1p)
            pev = pe[:, :].rearrange("p (qt g j) -> p qt g j",
                                     qt=2 * T, g=S, j=16)
            q1 = acc.tile([P, 2 * T * S * 8], b16)
            q1v = q1[:, :].rearrange("p (qt g j) -> p qt g j",
                                     qt=2 * T, g=S, j=8)
            nc.vector.tensor_add(out=q1v, in0=pev[:, :, :, 0:8],
                                 in1=pev[:, :, :, 8:16])
            q2 = acc.tile([P, 2 * T * S * 4], b16)
            q2v = q2[:, :].rearrange("p (qt g j) -> p qt g j",
                                     qt=2 * T, g=S, j=4)
            nc.vector.tensor_add(out=q2v, in0=q1v[:, :, :, 0:4],
                                 in1=q1v[:, :, :, 4:8])
            q3 = acc.tile([P, 2 * T * S * 2], b16)
            q3v = q3[:, :].rearrange("p (qt g j) -> p qt g j",
                                     qt=2 * T, g=S, j=2)
            nc.vector.tensor_add(out=q3v, in0=q2v[:, :, :, 0:2],
                                 in1=q2v[:, :, :, 2:4])
            sq = acc.tile([P, 2 * GP], f32)
            nc.vector.tensor_add(
                out=sq[:, :].rearrange("p (qt g) -> p qt g", qt=2 * T, g=S),
                in0=q3v[:, :, :, 0], in1=q3v[:, :, :, 1])
            s = acc.tile([P, GP], f32)
            nc.vector.tensor_add(out=s[:, :], in0=sq[:, 0:GP],
                                 in1=sq[:, GP:2 * GP])
            junk = acc.tile([P, 2 * GP], f32)
            nc.vector.tensor_mul(
                out=junk[:, :].rearrange("p (c tg) -> p c tg", c=2, tg=GP),
                in0=s[:, :].rearrange("p tg -> p tg").unsqueeze(1)
                .broadcast_to((P, 2, GP)),
                in1=w_sb[:, :].rearrange("p (c tg) -> p c tg", c=2, tg=GP))
            o_sb = acc.tile([P, 2], f32)
            nc.vector.reduce_sum(
                out=o_sb[:, :].rearrange("p c -> p c"),
                in_=junk[:, :].rearrange("p (c tg) -> p c tg", c=2, tg=GP),
                axis=mybir.AxisListType.X)
            nc.sync.dma_start(out=o_d, in_=o_sb[:, :])

    return nc


def _get_nc():
    if "nc" not in _cache:
        _cache["nc"] = _build_nc()
    return _cache["nc"]


def _shard6(m):
    """(N,16,16) f32 -> (CORES, T, 128, S, 16, 16) [c,t,p,g,r,c2] padded."""
    out = np.zeros((NP, 16, 16), np.float32)
    out[:N] = np.asarray(m, np.float32)
    return out.reshape(CORES, T, 128, S, 16, 16)


def _shard_vec(v):
    out = np.zeros((NP, 16), np.float32)
    out[:N] = np.asarray(v, np.float32).reshape(N, 16)
    return out.reshape(CORES, T, 128, S, 16)


def kernel(h0, cw0, mw0, cw1, mw1,
           msg0_r0, msg0_r1, msg0_r2,
           msg1_r0, msg1_r1, msg1_r2,
           w_pred, b_pred):
    from concourse.bass_utils import run_bass_kernel_spmd

    nc = _get_nc()
    if not _cache.get("split_done"):
        import concourse.mybir as mybir
        _split_multiwait(nc, mybir)
        _cache["split_done"] = True

    # md: [c, t, p, reg(2: AB|DE), m/q(2), g(7), j(16), k/i(16)]
    md = np.empty((CORES, T, 128, 2, 2, S, 16, 16), BF16)
    md[:, :, :, 0, 0] = _shard6(cw0[0])
    md[:, :, :, 0, 1] = _shard6(mw0[0])
    md[:, :, :, 1, 0] = _shard6(cw1[0]).transpose(0, 1, 2, 3, 5, 4)
    md[:, :, :, 1, 1] = _shard6(mw1[0]).transpose(0, 1, 2, 3, 5, 4)
    MDS = np.ascontiguousarray(md[:, 0].reshape(CORES, 128, 7168))
    # pairs: G=(t2, g) -> [c, pr, p, reg, m, t2, g, j, k]
    MDP = np.ascontiguousarray(
        md[:, 1:7].reshape(CORES, 3, 2, 128, 2, 2, S, 16, 16)
        .transpose(0, 1, 3, 4, 5, 2, 6, 7, 8).reshape(CORES, 3, 128, 14336))

    X = _shard_vec(np.asarray(h0, np.float32)[..., 0])        # c t p g k
    M0 = _shard_vec(np.asarray(msg0_r0, np.float32)[..., 0])
    M1 = _shard_vec(np.asarray(msg1_r0, np.float32)[..., 0])
    aux = np.empty((CORES, 128, 2352), BF16)
    aux[:, :, 0:112] = X[:, 0].reshape(CORES, 128, 112)
    aux[:, :, 112:224] = M0[:, 0].reshape(CORES, 128, 112)
    aux[:, :, 224:336] = M1[:, 0].reshape(CORES, 128, 112)
    XP = X[:, 1:7].reshape(CORES, 3, 2, 128, 112).transpose(0, 3, 1, 2, 4)
    M0P = M0[:, 1:7].reshape(CORES, 3, 2, 128, 112).transpose(0, 3, 1, 2, 4)
    M1P = M1[:, 1:7].reshape(CORES, 3, 2, 128, 112).transpose(0, 3, 1, 2, 4)
    blocks = np.concatenate(
        [np.stack([XP, M0P], axis=3).reshape(CORES, 128, 3, 448),
         M1P.reshape(CORES, 128, 3, 224)], axis=3)        # [c, p, 3, 672]
    aux[:, :, 336:2352] = blocks.reshape(CORES, 128, 2016)
    AUX = np.ascontiguousarray(aux)

    wp = np.zeros((2, NP), np.float32)
    wp[:, :N] = np.asarray(w_pred, np.float32)
    W = np.ascontiguousarray(
        wp.reshape(2, CORES, T, 128, S).transpose(1, 3, 0, 2, 4)
        .reshape(CORES, 128, 2 * GP))

    in_maps = [
        {"mds": MDS[i], "mdp": MDP[i], "aux": AUX[i], "w": W[i]}
        for i in range(CORES)
    ]
    res = run_bass_kernel_spmd(nc, in_maps, list(range(CORES)), trace=TRACE)
    _cache["last_res"] = res
    partial = np.zeros(2, np.float64)
    for i in range(CORES):
        partial += res.results[i]["o"].astype(np.float64).sum(axis=0)
    out = (partial + np.asarray(b_pred, np.float64)).astype(np.float32)
    return out.reshape(1, 2)
